# revision 1
# baseline (speedup 1.0000x reference)
"""Trainium2 Bass kernel for nn_BasicLayer (sparse cluster attention, 2 layers).

Strategy
--------
The scanline (boustrophedon) ordering commutes with every per-token op in the
network (LN, matmuls, residuals) and the attention operates on contiguous
64-token clusters *of the ordered sequence*.  So: gather x into curve order
once on the host, run both transformer layers on-device with attention over
contiguous clusters, and scatter back at the end.

Sharding: 65536 tokens total (4 batches x 16384) -> 8192 tokens per core
(half a batch each, aligned to cluster boundaries).  Weights replicated.

On-device layout: token-major fp32 residual stream (features padded 192->256
so layout flips are full 128x128 DMA-xbar transposes); bf16 feature-major
matmul operands.  LN affine and all biases are folded on the host.  Attention
runs per 128-token cluster-pair: per-head scores/O matmuls are K=128/N=128
full-pair blocks; cross-cluster leakage is killed by keeping the off-diagonal
blocks of the softmax matrix P permanently zero.
"""

import os
import numpy as np
import ml_dtypes

# ---- problem constants (hardcoded per contract) ----
B, N, D = 4, 16384, 192
DP = 256                             # padded feature dim for the residual
HEADS, DH, CLM = 6, 32, 64
GRID_W = 128
DEPTH = 2
NCORES = 8
T = (B * N) // NCORES                # 8192 tokens per core
SUB = 128
NSUB = 4
TILE = SUB * NSUB                    # 512-token supertile = 4 cluster pairs
NTILES = T // TILE                   # 16
DFF = 768

_COMPILED = {}


def _scanline_order(pos, w):
    ix = np.floor(pos[..., 0]).astype(np.int64)
    iy = np.floor(pos[..., 1]).astype(np.int64)
    key = iy * w + np.where(iy % 2 == 1, w - 1 - ix, ix)
    return np.argsort(key, axis=1, kind="stable")


def _fold_weights(inputs):
    """Fold LN affine + biases into matmul weights. Returns per-layer dicts of
    numpy arrays laid out exactly as the DRAM tensors the kernel declares."""
    bf16 = ml_dtypes.bfloat16
    scale = DH ** -0.5
    layers = []
    for i in range(DEPTH):
        g1 = np.asarray(inputs["ln1_g"][i], np.float64)
        b1 = np.asarray(inputs["ln1_b"][i], np.float64)
        Wqkv = np.asarray(inputs["w_qkv"][i], np.float64)
        bqkv = np.asarray(inputs["b_qkv"][i], np.float64)
        w_eff = g1[:, None] * Wqkv
        b_eff = b1 @ Wqkv + bqkv
        wq = w_eff[:, 0:D] * scale
        bq = b_eff[0:D] * scale
        wk = w_eff[:, D:2 * D]
        bk = b_eff[D:2 * D]
        wv = w_eff[:, 2 * D:3 * D]
        bv = b_eff[2 * D:3 * D]
        # qk weight M-layout: [q h0-3 | k h0-3 | q h4-5 | k h4-5]
        wqk = np.concatenate(
            [wq[:, :128], wk[:, :128], wq[:, 128:], wk[:, 128:]], axis=1)
        pad64 = np.zeros(64)
        bqk = np.stack(
            [bq[:128], bk[:128],
             np.concatenate([bq[128:], pad64]),
             np.concatenate([bk[128:], pad64])], axis=1)
        wp = np.asarray(inputs["w_proj"][i], np.float64)
        bp = np.asarray(inputs["b_proj"][i], np.float64)
        g2 = np.asarray(inputs["ln2_g"][i], np.float64)
        b2 = np.asarray(inputs["ln2_b"][i], np.float64)
        W1 = np.asarray(inputs["w_fc1"][i], np.float64)
        w1_eff = g2[:, None] * W1
        b1_eff = b2 @ W1 + np.asarray(inputs["b_fc1"][i], np.float64)
        W2 = np.asarray(inputs["w_fc2"][i], np.float64)
        bfc2 = np.asarray(inputs["b_fc2"][i], np.float64)
        bv_t = np.stack(
            [bv[:128], np.concatenate([bv[128:], np.zeros(64)])], axis=1)
        layers.append({
            f"wqk{i}": wqk.astype(bf16),
            f"bqk{i}": bqk.astype(np.float32),
            f"wv{i}": wv.astype(bf16),
            f"bv{i}": bv_t.astype(np.float32),
            f"wp{i}": wp.astype(bf16),
            f"bp{i}": np.tile(bp.astype(np.float32), (128, 1)),
            f"w1{i}": w1_eff.astype(bf16),
            f"b1{i}": b1_eff.reshape(6, 128).T.copy().astype(np.float32),
            f"w2{i}": W2.astype(bf16),
            f"b2{i}": np.tile(bfc2.astype(np.float32), (128, 1)),
        })
    return layers


def _build_nc(biases_zero=False):
    """Build + compile the Bass program (once per process)."""
    key = ("nc", biases_zero)
    if key in _COMPILED:
        return _COMPILED[key]

    from contextlib import ExitStack
    import concourse.bass as bass
    import concourse.tile as tile
    from concourse import bacc, mybir
    from concourse.bass import ts, ds

    f32 = mybir.dt.float32
    bf16 = mybir.dt.bfloat16
    AF = mybir.ActivationFunctionType
    OP = mybir.AluOpType

    nc = bacc.Bacc("TRN2", target_bir_lowering=False, debug=False,
                   enable_asserts=False, num_devices=NCORES)

    x_d = nc.dram_tensor("x", [T, DP], f32, kind="ExternalInput").ap()
    y_d = nc.dram_tensor("y", [T, D], f32, kind="ExternalOutput").ap()
    wd = []
    for i in range(DEPTH):
        wd.append({
            "wqk": nc.dram_tensor(f"wqk{i}", [D, 384], bf16, kind="ExternalInput").ap(),
            "bqk": nc.dram_tensor(f"bqk{i}", [128, 4], f32, kind="ExternalInput").ap(),
            "wv": nc.dram_tensor(f"wv{i}", [D, D], bf16, kind="ExternalInput").ap(),
            "bv": nc.dram_tensor(f"bv{i}", [128, 2], f32, kind="ExternalInput").ap(),
            "wp": nc.dram_tensor(f"wp{i}", [D, D], bf16, kind="ExternalInput").ap(),
            "bp": nc.dram_tensor(f"bp{i}", [128, D], f32, kind="ExternalInput").ap(),
            "w1": nc.dram_tensor(f"w1{i}", [D, DFF], bf16, kind="ExternalInput").ap(),
            "b1": nc.dram_tensor(f"b1{i}", [128, 6], f32, kind="ExternalInput").ap(),
            "w2": nc.dram_tensor(f"w2{i}", [DFF, D], bf16, kind="ExternalInput").ap(),
            "b2": nc.dram_tensor(f"b2{i}", [128, D], f32, kind="ExternalInput").ap(),
        })

    with tile.TileContext(nc) as tc, ExitStack() as ctx:
        consts = ctx.enter_context(tc.tile_pool(name="consts", bufs=1))
        xpool = ctx.enter_context(tc.tile_pool(name="xpool", bufs=3))
        wpool = ctx.enter_context(tc.tile_pool(name="wpool", bufs=3))
        spool = ctx.enter_context(tc.tile_pool(name="spool", bufs=3))
        stpool = ctx.enter_context(tc.tile_pool(name="stpool", bufs=6))
        pp_early = ctx.enter_context(tc.tile_pool(name="pp_early", bufs=2, space="PSUM"))
        pp_mid = ctx.enter_context(tc.tile_pool(name="pp_mid", bufs=4, space="PSUM"))
        pp_late = ctx.enter_context(tc.tile_pool(name="pp_late", bufs=2, space="PSUM"))

        from concourse.masks import make_identity
        ident = consts.tile([128, 128], bf16)
        make_identity(nc, ident)
        eps_t = consts.tile([128, 1], f32)
        nc.vector.memset(eps_t, 1e-5)
        # persistent softmax tiles: off-diagonal (cross-cluster) blocks stay 0
        NPBUF = 3
        p_bufs = []
        for pb_i in range(NPBUF):
            pb = consts.tile([128, HEADS, 128], bf16, name=f"pbuf{pb_i}")
            nc.vector.memset(pb, 0.0)
            p_bufs.append(pb)

        # --- load weights into SBUF once ---
        W = []
        for i in range(DEPTH):
            d = wd[i]
            sb = {}
            sb["wqk0"] = consts.tile([128, 384], bf16, name=f"wqk0{i}")
            sb["wqk1"] = consts.tile([64, 384], bf16, name=f"wqk1{i}")
            nc.sync.dma_start(out=sb["wqk0"], in_=d["wqk"][0:128])
            nc.sync.dma_start(out=sb["wqk1"], in_=d["wqk"][128:192])
            sb["wv0"] = consts.tile([128, D], bf16, name=f"wv0{i}")
            sb["wv1"] = consts.tile([64, D], bf16, name=f"wv1{i}")
            nc.sync.dma_start(out=sb["wv0"], in_=d["wv"][0:128])
            nc.sync.dma_start(out=sb["wv1"], in_=d["wv"][128:192])
            sb["wp0"] = consts.tile([128, D], bf16, name=f"wp0{i}")
            sb["wp1"] = consts.tile([64, D], bf16, name=f"wp1{i}")
            nc.sync.dma_start(out=sb["wp0"], in_=d["wp"][0:128])
            nc.sync.dma_start(out=sb["wp1"], in_=d["wp"][128:192])
            sb["w10"] = consts.tile([128, DFF], bf16, name=f"w10{i}")
            sb["w11"] = consts.tile([64, DFF], bf16, name=f"w11{i}")
            nc.sync.dma_start(out=sb["w10"], in_=d["w1"][0:128])
            nc.sync.dma_start(out=sb["w11"], in_=d["w1"][128:192])
            sb["w2m"] = consts.tile([128, 6, D], bf16, name=f"w2m{i}")
            nc.sync.dma_start(
                out=sb["w2m"],
                in_=d["w2"].rearrange("(m p) n -> p m n", p=128))
            for nm in ("bqk", "bv", "b1", "bp", "b2"):
                shp = {"bqk": [128, 4], "bv": [128, 2], "b1": [128, 6],
                       "bp": [128, D], "b2": [128, D]}[nm]
                sb[nm] = consts.tile(shp, f32, name=f"{nm}{i}")
                nc.sync.dma_start(out=sb[nm], in_=d[nm])
            W.append(sb)

        pair_ctr = [0]

        def layernorm_to_fm(x_t, tag):
            """LN on token-major x_t -> bf16 feature-major chunk tiles
            fmA (feats 0:128) and fmA2 (feats 128:256; rows 64: are pad)."""
            xn = wpool.tile([128, NSUB, DP], bf16, tag=f"xn{tag}", name=f"xn{tag}")
            mv4 = stpool.tile([128, NSUB, 2], f32, tag="mv", name="mv4")
            sd4 = stpool.tile([128, NSUB], f32, tag="sd", name="sd4")
            r4 = stpool.tile([128, NSUB], f32, tag="r", name="r4")
            for s in range(NSUB):
                st = stpool.tile([128, 6], f32, tag="st", name="st")
                nc.vector.bn_stats(st, x_t[:, s, 0:D])
                nc.vector.bn_aggr(mv4[:, s], st)
            nc.scalar.activation(sd4, mv4[:, :, 1], AF.Sqrt, bias=eps_t)
            nc.vector.reciprocal(r4, sd4)
            fmA = wpool.tile([128, TILE], bf16, tag=f"fmA{tag}", name=f"fmA{tag}")
            fmA2 = wpool.tile([128, TILE], bf16, tag=f"fmA2{tag}", name=f"fmA2{tag}")
            psA = pp_early.tile([128, TILE], bf16, tag="early", name="psA")
            psA2 = pp_early.tile([128, TILE], bf16, tag="early", name="psA2")
            for s in range(NSUB):
                lnt = stpool.tile([128, DP], f32, tag="lnt", name="lnt")
                mus = mv4[:, s, 0:1]
                mu_b = bass.AP(tensor=mus.tensor, offset=mus.offset,
                               ap=[mus.ap[0], [0, DP]])
                rs = r4[:, s:s + 1]
                r_b = bass.AP(tensor=rs.tensor, offset=rs.offset,
                              ap=[rs.ap[0], [0, DP]])
                nc.gpsimd.tensor_tensor(out=lnt, in0=x_t[:, s], in1=mu_b,
                                        op=OP.subtract)
                nc.gpsimd.tensor_tensor(out=xn[:, s], in0=lnt, in1=r_b,
                                        op=OP.mult)
                nc.tensor.transpose(psA[:, ts(s, 128)], xn[:, s, 0:128], ident)
                nc.tensor.transpose(psA2[:, ts(s, 128)], xn[:, s, 128:256], ident)
            nc.vector.tensor_copy(fmA, psA)
            nc.vector.tensor_copy(fmA2, psA2)
            return fmA, fmA2

        def mlp_block(sb, ynA, ynA2, x_t):
            hfm = wpool.tile([128, 6, TILE], bf16, tag="hfm", name="hfm")
            for m in range(6):
                ps = pp_late.tile([128, TILE], f32, tag="late", name="psh")
                nc.tensor.matmul(ps, sb["w10"][:, ts(m, 128)], ynA,
                                 start=True, stop=False)
                nc.tensor.matmul(ps, sb["w11"][:, ts(m, 128)], ynA2[0:64],
                                 start=False, stop=True)
                nc.scalar.activation(hfm[:, m], ps, AF.Gelu_apprx_tanh,
                                     bias=sb["b1"][:, m:m + 1])
            for s in range(NSUB):
                ps = pp_late.tile([128, D], f32, tag="late", name="psf2")
                for m in range(6):
                    nc.tensor.matmul(ps, hfm[:, m, ts(s, 128)],
                                     sb["w2m"][:, m],
                                     start=(m == 0), stop=(m == 5))
                nc.vector.tensor_add(x_t[:, s, 0:D], x_t[:, s, 0:D], ps)
                if not biases_zero:
                    nc.vector.tensor_add(x_t[:, s, 0:D], x_t[:, s, 0:D],
                                         sb["b2"])

        for it in range(NTILES):
            x_t = xpool.tile([128, NSUB, DP], f32, tag="x", name="x_t")
            nc.sync.dma_start(
                out=x_t,
                in_=x_d[ts(it, TILE)].rearrange("(s p) f -> p s f", p=128))

            for li in range(DEPTH):
                sb = W[li]
                # ---- LN1 -> feature-major ----
                xnA, xnA2 = layernorm_to_fm(x_t, "1")
                # ---- q,k (feature-major out) ----
                qA = wpool.tile([128, TILE], bf16, tag="qA", name="qA")
                kA = wpool.tile([128, TILE], bf16, tag="kA", name="kA")
                qB = wpool.tile([64, TILE], bf16, tag="qB", name="qB")
                kB = wpool.tile([64, TILE], bf16, tag="kB", name="kB")
                for m in range(4):
                    mw = 128 if m < 2 else 64
                    moff = m * 128 if m < 2 else 256 + (m - 2) * 64
                    ps = pp_early.tile([mw, TILE], f32, tag="early", name="psqk")
                    nc.tensor.matmul(ps, sb["wqk0"][:, ds(moff, mw)], xnA,
                                     start=True, stop=False)
                    nc.tensor.matmul(ps, sb["wqk1"][:, ds(moff, mw)],
                                     xnA2[0:64], start=False, stop=True)
                    dst = (qA, kA, qB, kB)[m]
                    if biases_zero:
                        if m % 2 == 0:
                            nc.vector.tensor_copy(dst, ps)
                        else:
                            nc.scalar.activation(dst, ps, AF.Copy)
                    else:
                        nc.scalar.activation(dst, ps, AF.Identity,
                                             bias=sb["bqk"][0:mw, m:m + 1])
                # per-head base-0 copies (PE row tile position must be 0)
                q6 = wpool.tile([32, HEADS, TILE], bf16, tag="q6", name="q6")
                k6 = wpool.tile([32, HEADS, TILE], bf16, tag="k6", name="k6")
                for h in range(HEADS):
                    qsrc = qA[ts(h, 32)] if h < 4 else qB[ts(h - 4, 32)]
                    ksrc = kA[ts(h, 32)] if h < 4 else kB[ts(h - 4, 32)]
                    nc.gpsimd.dma_start(out=q6[:, h], in_=qsrc)
                    nc.gpsimd.dma_start(out=k6[:, h], in_=ksrc)
                # ---- v (token-major out) ----
                v_tm = wpool.tile([128, NSUB, D], bf16, tag="vtm", name="v_tm")
                for s in range(NSUB):
                    ps = pp_mid.tile([128, D], f32, tag="mid", name="psv")
                    nc.tensor.matmul(ps, xnA[:, ts(s, 128)], sb["wv0"],
                                     start=True, stop=False)
                    nc.tensor.matmul(ps, xnA2[0:64, ts(s, 128)], sb["wv1"],
                                     start=False, stop=True)
                    if s % 2 == 0:
                        nc.vector.tensor_copy(v_tm[:, s], ps)
                    else:
                        nc.scalar.activation(v_tm[:, s], ps, AF.Copy)
                # ---- attention: one cluster-pair (128 tokens) per sub ----
                ofmA = wpool.tile([128, TILE], bf16, tag="ofA", name="ofmA")
                ofmB = wpool.tile([64, TILE], bf16, tag="ofB", name="ofmB")
                for s in range(NSUB):
                    scA = pp_mid.tile([128, 3, 128], f32, tag="mid", name="scA")
                    scB = pp_mid.tile([128, 3, 128], f32, tag="mid", name="scB")
                    for h in range(HEADS):
                        sct = scA if h < 3 else scB
                        cols = ds(s * 128, 128)
                        nc.tensor.matmul(sct[:, h % 3], q6[:, h, cols],
                                         k6[:, h, cols],
                                         start=True, stop=True)
                    E = spool.tile([128, HEADS, 128], bf16, tag="E", name="E")
                    sums = stpool.tile([128, HEADS], f32, tag="sm", name="sums")
                    rsum = stpool.tile([128, HEADS], f32, tag="rs", name="rsum")
                    nc.scalar.activation(E[:, 0:3], scA, AF.Exp)
                    nc.scalar.activation(E[:, 3:6], scB, AF.Exp)
                    nc.vector.reduce_sum(sums[0:64], E[0:64, :, 0:64],
                                         axis=mybir.AxisListType.X)
                    nc.vector.reduce_sum(sums[64:128], E[64:128, :, 64:128],
                                         axis=mybir.AxisListType.X)
                    nc.vector.reciprocal(rsum, sums)
                    P = p_bufs[pair_ctr[0] % NPBUF]
                    pair_ctr[0] += 1
                    for half in range(2):
                        hs = ds(half * 64, 64)
                        rs_half = rsum[ds(half * 64, 64)]
                        rsum_b = bass.AP(tensor=rs_half.tensor,
                                         offset=rs_half.offset,
                                         ap=[*rs_half.ap, [0, 64]])
                        nc.gpsimd.tensor_tensor(
                            out=P[hs, :, hs], in0=E[hs, :, hs],
                            in1=rsum_b, op=OP.mult)
                    pT = pp_mid.tile([128, HEADS, 128], bf16, tag="mid", name="pT")
                    for h in range(HEADS):
                        nc.tensor.transpose(pT[:, h], P[:, h], ident)
                    pkm = spool.tile([128, HEADS, 128], bf16, tag="pkm",
                                     name="pkm")
                    if s % 2 == 0:
                        nc.vector.tensor_copy(pkm, pT)
                    else:
                        nc.scalar.activation(pkm, pT, AF.Copy)
                    oP = pp_mid.tile([128, 256], f32, tag="mid", name="oP")
                    for h in range(HEADS):
                        if h < 4:
                            out = oP[ts(h, 32), 0:128]
                            colpos = h * 32
                        else:
                            out = oP[ts(h - 4, 32), 128:256]
                            colpos = (h - 4) * 32
                        nc.tensor.matmul(out, v_tm[:, s, ts(h, 32)],
                                         pkm[:, h], start=True, stop=True,
                                         tile_position=(0, colpos))
                    if biases_zero:
                        nc.scalar.activation(ofmA[:, ts(s, 128)],
                                             oP[:, 0:128], AF.Copy)
                        nc.vector.tensor_copy(ofmB[:, ts(s, 128)],
                                              oP[0:64, 128:256])
                    else:
                        nc.scalar.activation(ofmA[:, ts(s, 128)],
                                             oP[:, 0:128], AF.Identity,
                                             bias=sb["bv"][:, 0:1])
                        nc.scalar.activation(ofmB[:, ts(s, 128)],
                                             oP[0:64, 128:256],
                                             AF.Identity,
                                             bias=sb["bv"][0:64, 1:2])
                # ---- proj + residual ----
                for s in range(NSUB):
                    ps = pp_late.tile([128, D], f32, tag="late", name="psp")
                    nc.tensor.matmul(ps, ofmA[:, ts(s, 128)], sb["wp0"],
                                     start=True, stop=False)
                    nc.tensor.matmul(ps, ofmB[:, ts(s, 128)], sb["wp1"],
                                     start=False, stop=True)
                    nc.vector.tensor_add(x_t[:, s, 0:D], x_t[:, s, 0:D], ps)
                    if not biases_zero:
                        nc.vector.tensor_add(x_t[:, s, 0:D], x_t[:, s, 0:D],
                                             sb["bp"])
                # ---- LN2 + MLP ----
                ynA, ynA2 = layernorm_to_fm(x_t, "2")
                mlp_block(sb, ynA, ynA2, x_t)

            nc.sync.dma_start(
                out=y_d[ts(it, TILE)].rearrange("(s p) f -> p s f", p=128),
                in_=x_t[:, :, 0:D])

    nc.compile()
    _COMPILED[key] = nc
    return nc


def _ensure_ntff_hook():
    """The image's antenv package lacks axon_hooks; synthesize it and install
    the ctypes-based NTFF profile hook from trn_agent_boot (test-only path)."""
    import sys, types
    if "antenv.axon_hooks" in sys.modules:
        return True
    try:
        mod = types.ModuleType("antenv.axon_hooks")
        state = {}
        mod.set_axon_ntff_profile_hook = lambda h: state.__setitem__("h", h)
        mod.get_axon_ntff_profile_hook = lambda: state.get("h")
        sys.modules["antenv.axon_hooks"] = mod
        import antenv
        antenv.axon_hooks = mod
        from trn_agent_boot.trn_boot import _ntff_profile_via_ctypes
        mod.set_axon_ntff_profile_hook(
            _ntff_profile_via_ctypes("/opt/axon/libaxon_pjrt.so"))
        return True
    except Exception as e:  # pragma: no cover
        print(f"NTFF hook shim failed: {e}")
        return False


def _run(inputs, trace=False):
    """Shard, execute on 8 cores, gather. Returns (y_full, exec_time_ns)."""
    from concourse.bass_utils import run_bass_kernel_spmd

    if trace:
        trace = _ensure_ntff_hook()

    layers = _fold_weights(inputs)
    bz = all(
        not np.any(np.asarray(d[k], np.float32))
        for d in layers for k in d if k.startswith(("bp", "b2")))
    nc = _build_nc(biases_zero=bz)

    x = np.asarray(inputs["x"], np.float32)
    pos = np.asarray(inputs["pos"], np.float32)
    w = int(np.asarray(inputs["w"]))
    order = _scanline_order(pos, w)
    x_ord = np.take_along_axis(x, order[..., None], axis=1)
    shards = np.zeros((NCORES, T, DP), np.float32)
    shards[:, :, 0:D] = x_ord.reshape(NCORES, T, D)

    wmap = {}
    for d in layers:
        wmap.update({k: np.ascontiguousarray(v) for k, v in d.items()})

    in_maps = [{"x": shards[c], **wmap} for c in range(NCORES)]
    res = run_bass_kernel_spmd(nc, in_maps, core_ids=list(range(NCORES)),
                               trace=trace)
    y_ord = np.stack([res.results[c]["y"] for c in range(NCORES)])
    y_ord = y_ord.reshape(B, N, D)
    y = np.empty_like(y_ord)
    np.put_along_axis(y, order[..., None], y_ord, axis=1)
    return y.astype(np.float32), res.exec_time_ns


def kernel(**inputs):
    y, _ = _run(inputs, trace=False)
    return y



# revision 4
# speedup vs baseline: 1.3434x; 1.3434x over previous
"""Trainium2 Bass kernel for nn_BasicLayer (sparse cluster attention, 2 layers).

v2 rewrite of the staged baseline. Same host-side strategy (scanline gather,
8 cores x 8192 tokens, folded weights, token-major fp32 residual, bf16 matmul
operands) with an on-device restructure aimed at engine balance and PE
density:

- All layout flips (LN token-major -> feature-major, P -> P^T) go through the
  DMA xbar transpose engine instead of PE transpose + PSUM copy.
- LN: one batched bn_stats pair, rsqrt via fast-inverse-sqrt bit trick +
  1 Newton step (DVE only, no sqrt table), normalize via dual-scalar
  tensor_scalar producing bf16 directly.
- Scores matmuls read per-head q/k slices in place via tile_position row
  packing (no per-head copies).
- Softmax: exp -> scratch E; P = E * (1/rowsum) only on the diagonal
  64x64 blocks into persistent zeroed P buffers (gpsimd); P^T via DMA
  transpose feeds the O matmuls.
- Supertiles processed in groups of 4 with phase-major ordering per layer so
  the scalar engine's activation-table switches (Exp <-> Gelu) amortize
  across the group.
"""

import os
import numpy as np
import ml_dtypes

# ---- problem constants (hardcoded per contract) ----
B, N, D = 4, 16384, 192
DP = 256
HEADS, DH, CLM = 6, 32, 64
GRID_W = 128
DEPTH = 2
NCORES = 8
T = (B * N) // NCORES                # 8192 tokens per core
SUB = 128
NSUB = 4
TILE = SUB * NSUB                    # 512-token supertile
NTILES = T // TILE                   # 16
GROUP = 4                            # supertiles per phase group
DFF = 768

_COMPILED = {}


def _scanline_order(pos, w):
    ix = np.floor(pos[..., 0]).astype(np.int64)
    iy = np.floor(pos[..., 1]).astype(np.int64)
    key = iy * w + np.where(iy % 2 == 1, w - 1 - ix, ix)
    return np.argsort(key, axis=1, kind="stable")


def _fold_weights(inputs):
    """Fold LN affine + biases into matmul weights (same layout as v1)."""
    bf16 = ml_dtypes.bfloat16
    scale = DH ** -0.5
    layers = []
    for i in range(DEPTH):
        g1 = np.asarray(inputs["ln1_g"][i], np.float64)
        b1 = np.asarray(inputs["ln1_b"][i], np.float64)
        Wqkv = np.asarray(inputs["w_qkv"][i], np.float64)
        bqkv = np.asarray(inputs["b_qkv"][i], np.float64)
        w_eff = g1[:, None] * Wqkv
        b_eff = b1 @ Wqkv + bqkv
        wq = w_eff[:, 0:D] * scale
        bq = b_eff[0:D] * scale
        wk = w_eff[:, D:2 * D]
        bk = b_eff[D:2 * D]
        wv = w_eff[:, 2 * D:3 * D]
        bv = b_eff[2 * D:3 * D]
        wqk = np.concatenate(
            [wq[:, :128], wk[:, :128], wq[:, 128:], wk[:, 128:]], axis=1)
        pad64 = np.zeros(64)
        bqk = np.stack(
            [bq[:128], bk[:128],
             np.concatenate([bq[128:], pad64]),
             np.concatenate([bk[128:], pad64])], axis=1)
        wp = np.asarray(inputs["w_proj"][i], np.float64)
        bp = np.asarray(inputs["b_proj"][i], np.float64)
        g2 = np.asarray(inputs["ln2_g"][i], np.float64)
        b2 = np.asarray(inputs["ln2_b"][i], np.float64)
        W1 = np.asarray(inputs["w_fc1"][i], np.float64)
        w1_eff = g2[:, None] * W1
        b1_eff = b2 @ W1 + np.asarray(inputs["b_fc1"][i], np.float64)
        W2 = np.asarray(inputs["w_fc2"][i], np.float64)
        bfc2 = np.asarray(inputs["b_fc2"][i], np.float64)
        bv_t = np.stack(
            [bv[:128], np.concatenate([bv[128:], np.zeros(64)])], axis=1)
        layers.append({
            f"wqk{i}": wqk.astype(bf16),
            f"bqk{i}": bqk.astype(np.float32),
            f"wv{i}": wv.astype(bf16),
            f"bv{i}": bv_t.astype(np.float32),
            f"wp{i}": wp.astype(bf16),
            f"bp{i}": np.tile(bp.astype(np.float32), (128, 1)),
            f"w1{i}": w1_eff.astype(bf16),
            f"b1{i}": b1_eff.reshape(6, 128).T.copy().astype(np.float32),
            f"w2{i}": W2.astype(bf16),
            f"b2{i}": np.tile(bfc2.astype(np.float32), (128, 1)),
        })
    return layers


def _build_nc(biases_zero=True, ntiles=NTILES):
    key = ("nc", biases_zero, ntiles)
    if key in _COMPILED:
        return _COMPILED[key]

    from contextlib import ExitStack
    import concourse.bass as bass
    import concourse.tile as tile
    from concourse import bacc, mybir
    from concourse.bass import ts, ds

    f32 = mybir.dt.float32
    bf16 = mybir.dt.bfloat16
    i32 = mybir.dt.int32
    AF = mybir.ActivationFunctionType
    OP = mybir.AluOpType

    tok_total = ntiles * TILE

    nc = bacc.Bacc("TRN2", target_bir_lowering=False, debug=False,
                   enable_asserts=False, num_devices=NCORES)

    x_d = nc.dram_tensor("x", [tok_total, DP], f32, kind="ExternalInput").ap()
    y_d = nc.dram_tensor("y", [tok_total, D], f32, kind="ExternalOutput").ap()
    wd = []
    for i in range(DEPTH):
        wd.append({
            "wqk": nc.dram_tensor(f"wqk{i}", [D, 384], bf16, kind="ExternalInput").ap(),
            "bqk": nc.dram_tensor(f"bqk{i}", [128, 4], f32, kind="ExternalInput").ap(),
            "wv": nc.dram_tensor(f"wv{i}", [D, D], bf16, kind="ExternalInput").ap(),
            "bv": nc.dram_tensor(f"bv{i}", [128, 2], f32, kind="ExternalInput").ap(),
            "wp": nc.dram_tensor(f"wp{i}", [D, D], bf16, kind="ExternalInput").ap(),
            "bp": nc.dram_tensor(f"bp{i}", [128, D], f32, kind="ExternalInput").ap(),
            "w1": nc.dram_tensor(f"w1{i}", [D, DFF], bf16, kind="ExternalInput").ap(),
            "b1": nc.dram_tensor(f"b1{i}", [128, 6], f32, kind="ExternalInput").ap(),
            "w2": nc.dram_tensor(f"w2{i}", [DFF, D], bf16, kind="ExternalInput").ap(),
            "b2": nc.dram_tensor(f"b2{i}", [128, D], f32, kind="ExternalInput").ap(),
        })

    with tile.TileContext(nc) as tc, ExitStack() as ctx:
        consts = ctx.enter_context(tc.tile_pool(name="consts", bufs=1))
        xpool = ctx.enter_context(tc.tile_pool(name="xpool", bufs=6))
        lnpool = ctx.enter_context(tc.tile_pool(name="lnpool", bufs=3))
        fmpool = ctx.enter_context(tc.tile_pool(name="fmpool", bufs=4))
        qkpool = ctx.enter_context(tc.tile_pool(name="qkpool", bufs=5))
        apool = ctx.enter_context(tc.tile_pool(name="apool", bufs=3))
        ofpool = ctx.enter_context(tc.tile_pool(name="ofpool", bufs=5))
        hpool = ctx.enter_context(tc.tile_pool(name="hpool", bufs=2))
        stpool = ctx.enter_context(tc.tile_pool(name="stpool", bufs=6))
        ppb = ctx.enter_context(tc.tile_pool(name="ppb", bufs=1, space="PSUM"))
        ppsc = ctx.enter_context(tc.tile_pool(name="ppsc", bufs=1, space="PSUM"))
        ppm = ctx.enter_context(tc.tile_pool(name="ppm", bufs=2, space="PSUM"))

        # persistent softmax buffers: off-diagonal blocks stay 0 forever
        NPBUF = 4
        p_bufs = []
        for pb_i in range(NPBUF):
            pb = consts.tile([128, HEADS, 128], bf16, name=f"pbuf{pb_i}")
            nc.vector.memset(pb, 0.0)
            p_bufs.append(pb)

        # --- load weights into SBUF once ---
        W = []
        for i in range(DEPTH):
            d = wd[i]
            sb = {}
            sb["wqk0"] = consts.tile([128, 384], bf16, name=f"wqk0{i}")
            sb["wqk1"] = consts.tile([64, 384], bf16, name=f"wqk1{i}")
            nc.sync.dma_start(out=sb["wqk0"], in_=d["wqk"][0:128])
            nc.sync.dma_start(out=sb["wqk1"], in_=d["wqk"][128:192])
            sb["wv0"] = consts.tile([128, D], bf16, name=f"wv0{i}")
            sb["wv1"] = consts.tile([64, D], bf16, name=f"wv1{i}")
            nc.sync.dma_start(out=sb["wv0"], in_=d["wv"][0:128])
            nc.sync.dma_start(out=sb["wv1"], in_=d["wv"][128:192])
            sb["wp0"] = consts.tile([128, D], bf16, name=f"wp0{i}")
            sb["wp1"] = consts.tile([64, D], bf16, name=f"wp1{i}")
            nc.sync.dma_start(out=sb["wp0"], in_=d["wp"][0:128])
            nc.sync.dma_start(out=sb["wp1"], in_=d["wp"][128:192])
            sb["w10"] = consts.tile([128, DFF], bf16, name=f"w10{i}")
            sb["w11"] = consts.tile([64, DFF], bf16, name=f"w11{i}")
            nc.sync.dma_start(out=sb["w10"], in_=d["w1"][0:128])
            nc.sync.dma_start(out=sb["w11"], in_=d["w1"][128:192])
            sb["w2m"] = consts.tile([128, 6, D], bf16, name=f"w2m{i}")
            nc.sync.dma_start(
                out=sb["w2m"],
                in_=d["w2"].rearrange("(m p) n -> p m n", p=128))
            for nm in ("bqk", "bv", "b1", "bp", "b2"):
                shp = {"bqk": [128, 4], "bv": [128, 2], "b1": [128, 6],
                       "bp": [128, D], "b2": [128, D]}[nm]
                sb[nm] = consts.tile(shp, f32, name=f"{nm}{i}")
                nc.sync.dma_start(out=sb[nm], in_=d[nm])
            W.append(sb)

        pair_ctr = [0]
        MAGIC = 0x5F3759DF
        # CoreSim lacks Gelu_apprx_tanh; substitute Tanh for sim-only runs.
        GELU_FUNC = (AF.Tanh if os.environ.get("K_SIM_GELU_TANH") == "1"
                     else AF.Gelu_apprx_tanh)

        def layernorm_fm(x_t, tag):
            """LN on token-major x_t -> feature-major bf16 via DMA transpose.
            Returns fmA [128,4,128] (feats 0:128, cols=tokens) and fmA2
            (feats 128:256; partitions 64:128 are pad)."""
            mv = stpool.tile([128, 4, 6], f32, tag="mv", name="mv")
            mv2 = stpool.tile([128, 4, 2], f32, tag="mv2", name="mv2")
            for s in range(NSUB):
                nc.vector.bn_stats(mv[:, s], x_t[:, s, 0:D])
                nc.vector.bn_aggr(mv2[:, s], mv[:, s])
            var = mv2[:, :, 1]                       # [128, 4] stride 2
            t_i = stpool.tile([128, 4], i32, tag="ti", name="t_i")
            y0 = stpool.tile([128, 4], f32, tag="y0", name="y0")
            zz = stpool.tile([128, 4], f32, tag="zz", name="zz")
            r4 = stpool.tile([128, 4], f32, tag="r4", name="r4")
            nc.vector.tensor_scalar(
                out=t_i, in0=var.bitcast(i32), scalar1=1, scalar2=None,
                op0=OP.logical_shift_right)
            nc.vector.tensor_scalar(
                out=y0.bitcast(i32), in0=t_i, scalar1=MAGIC, scalar2=-1,
                op0=OP.subtract, op1=OP.mult)
            nc.vector.scalar_tensor_tensor(
                out=zz, in0=var, scalar=1e-5, in1=y0,
                op0=OP.add, op1=OP.mult)              # (var+eps)*y0
            nc.vector.tensor_tensor(out=zz, in0=zz, in1=y0, op=OP.mult)
            nc.vector.tensor_scalar(
                out=zz, in0=zz, scalar1=-0.5, scalar2=1.5,
                op0=OP.mult, op1=OP.add)              # 1.5 - 0.5 v y0^2
            nc.vector.tensor_tensor(out=r4, in0=zz, in1=y0, op=OP.mult)

            xn = lnpool.tile([128, 2, NSUB, 128], bf16, tag=f"xn{tag}",
                             name=f"xn{tag}")
            for s in range(NSUB):
                nc.vector.tensor_scalar(
                    out=xn[:, :, s], in0=x_t[:, s].rearrange("p (c f) -> p c f", c=2),
                    scalar1=mv2[:, s, 0:1], scalar2=r4[:, s:s + 1],
                    op0=OP.subtract, op1=OP.mult)
            fmA = fmpool.tile([128, NSUB, 128], bf16, tag=f"fmA{tag}",
                              name=f"fmA{tag}")
            fmA2 = fmpool.tile([128, NSUB, 128], bf16, tag=f"fmA2{tag}",
                               name=f"fmA2{tag}")
            nc.sync.dma_start_transpose(out=fmA, in_=xn[:, 0])
            nc.sync.dma_start_transpose(out=fmA2, in_=xn[:, 1])
            return fmA, fmA2

        def phase_a(sb, x_t):
            """LN1 + qkv + v. Returns (qkA, qkB, v_tm)."""
            fmA, fmA2 = layernorm_fm(x_t, "1")
            fmAf = fmA.rearrange("p a b -> p (a b)")
            fmA2f = fmA2.rearrange("p a b -> p (a b)")
            # qkv: m-chunks 0,1 are 128-wide (heads 0-3 q|k), 2,3 are 64-wide
            psA = ppb.tile([128, 2, TILE], f32, tag="big", name="psqkA")
            psB = ppb.tile([64, 2, TILE], f32, tag="big", name="psqkB")
            for m in range(2):
                nc.tensor.matmul(psA[:, m], sb["wqk0"][:, ts(m, 128)], fmAf,
                                 start=True, stop=False)
                nc.tensor.matmul(psA[:, m], sb["wqk1"][:, ts(m, 128)],
                                 fmA2f[0:64], start=False, stop=True)
            for m in range(2):
                nc.tensor.matmul(psB[:, m], sb["wqk0"][:, ds(256 + m * 64, 64)],
                                 fmAf, start=True, stop=False)
                nc.tensor.matmul(psB[:, m], sb["wqk1"][:, ds(256 + m * 64, 64)],
                                 fmA2f[0:64], start=False, stop=True)
            qkA = qkpool.tile([128, 2, TILE], bf16, tag="qkA", name="qkA")
            qkB = qkpool.tile([64, 2, TILE], bf16, tag="qkB", name="qkB")
            if biases_zero:
                nc.vector.tensor_copy(qkA, psA)
                nc.vector.tensor_copy(qkB, psB)
            else:
                for m in range(2):
                    nc.scalar.activation(qkA[:, m], psA[:, m], AF.Identity,
                                         bias=sb["bqk"][:, m:m + 1])
                    nc.scalar.activation(qkB[:, m], psB[:, m], AF.Identity,
                                         bias=sb["bqk"][0:64, 2 + m:3 + m])
            # v (token-major out), pairs of subs per PSUM tile
            v_tm = qkpool.tile([128, NSUB, D], bf16, tag="vtm", name="v_tm")
            for sp in range(2):
                psv = ppm.tile([128, 2, 256], f32, tag="med", name="psv")
                for j in range(2):
                    s = sp * 2 + j
                    nc.tensor.matmul(psv[:, j, 0:D], fmA[:, s], sb["wv0"],
                                     start=True, stop=False)
                    nc.tensor.matmul(psv[:, j, 0:D], fmA2[0:64, s], sb["wv1"],
                                     start=False, stop=True)
                if biases_zero:
                    nc.scalar.activation(v_tm[:, ds(sp * 2, 2)],
                                         psv[:, :, 0:D], AF.Copy)
                else:
                    for j in range(2):
                        nc.scalar.activation(
                            v_tm[:, sp * 2 + j], psv[:, j, 0:D], AF.Identity,
                            bias=sb["bv"][:, 0:1])
            return qkA, qkB, v_tm

        def phase_b(sb, qkA, qkB, v_tm):
            """Cluster attention. Returns (ofmA, ofmB) feature-major o."""
            ofmA = ofpool.tile([128, TILE], bf16, tag="ofA", name="ofmA")
            ofmB = ofpool.tile([64, TILE], bf16, tag="ofB", name="ofmB")
            for sp in range(2):
                oPp = ppm.tile([128, 2, 256], f32, tag="med", name="oPp")
                for j in range(2):
                    s = sp * 2 + j
                    cols = ts(s, 128)
                    # concurrent row-tiled score MMs must land in distinct
                    # PSUM banks: head h -> bank h%4, col block h//4
                    sc = ppsc.tile([128, 4, 512], f32, tag="sc", name="sc")
                    for h in range(HEADS):
                        if h < 4:
                            qs = qkA[ts(h, 32), 0, cols]
                            ks = qkA[ts(h, 32), 1, cols]
                        else:
                            qs = qkB[ts(h - 4, 32), 0, cols]
                            ks = qkB[ts(h - 4, 32), 1, cols]
                        out = sc[:, h % 4, ds((h // 4) * 128, 128)]
                        nc.tensor.matmul(out, qs, ks,
                                         start=True, stop=True,
                                         tile_position=(32 * (h % 4), 0))
                    E = apool.tile([128, HEADS, 128], bf16, tag="E", name="E")
                    sums = stpool.tile([128, HEADS], f32, tag="sm", name="sums")
                    rsum = stpool.tile([128, HEADS], f32, tag="rs", name="rsum")
                    nc.scalar.activation(E[:, 0:4], sc[:, :, 0:128], AF.Exp)
                    nc.scalar.activation(E[:, 4:6], sc[:, 0:2, 128:256], AF.Exp)
                    nc.vector.reduce_sum(sums[0:64], E[0:64, :, 0:64],
                                         axis=mybir.AxisListType.X)
                    nc.vector.reduce_sum(sums[64:128], E[64:128, :, 64:128],
                                         axis=mybir.AxisListType.X)
                    nc.vector.reciprocal(rsum, sums)
                    P = p_bufs[pair_ctr[0] % NPBUF]
                    pair_ctr[0] += 1
                    for half in range(2):
                        hs = ds(half * 64, 64)
                        rs_half = rsum[ds(half * 64, 64)]
                        rsum_b = bass.AP(tensor=rs_half.tensor,
                                         offset=rs_half.offset,
                                         ap=[*rs_half.ap, [0, 64]])
                        nc.gpsimd.tensor_tensor(
                            out=P[hs, :, hs], in0=E[hs, :, hs],
                            in1=rsum_b, op=OP.mult)
                    pkm = apool.tile([128, HEADS, 128], bf16, tag="pkm",
                                     name="pkm")
                    nc.sync.dma_start_transpose(out=pkm, in_=P)
                    for h in range(HEADS):
                        if h < 4:
                            out = oPp[ts(h, 32), j, 0:128]
                            colpos = h * 32
                        else:
                            out = oPp[ts(h - 4, 32), j, 128:256]
                            colpos = (h - 4) * 32
                        nc.tensor.matmul(out, v_tm[:, s, ts(h, 32)],
                                         pkm[:, h], start=True, stop=True,
                                         tile_position=(0, colpos))
                # copy pair of subs out of PSUM
                c0 = sp * 256
                if biases_zero:
                    nc.vector.tensor_copy(
                        ofmA.rearrange("p (a b) -> p a b", a=NSUB)[:, ds(sp * 2, 2)],
                        oPp[:, :, 0:128])
                    nc.scalar.activation(
                        ofmB.rearrange("p (a b) -> p a b", a=NSUB)[:, ds(sp * 2, 2)],
                        oPp[0:64, :, 128:256], AF.Copy)
                else:
                    for j in range(2):
                        nc.scalar.activation(
                            ofmA[:, ds(c0 + j * 128, 128)], oPp[:, j, 0:128],
                            AF.Identity, bias=sb["bv"][:, 0:1])
                        nc.scalar.activation(
                            ofmB[:, ds(c0 + j * 128, 128)],
                            oPp[0:64, j, 128:256],
                            AF.Identity, bias=sb["bv"][0:64, 1:2])
            return ofmA, ofmB

        def phase_c(sb, x_t, ofmA, ofmB):
            """proj + residual, LN2, MLP + residual."""
            for sp in range(2):
                psp = ppm.tile([128, 2, 256], f32, tag="med", name="psp")
                for j in range(2):
                    s = sp * 2 + j
                    nc.tensor.matmul(psp[:, j, 0:D], ofmA[:, ts(s, 128)],
                                     sb["wp0"], start=True, stop=False)
                    nc.tensor.matmul(psp[:, j, 0:D], ofmB[:, ts(s, 128)],
                                     sb["wp1"], start=False, stop=True)
                nc.vector.tensor_add(x_t[:, ds(sp * 2, 2), 0:D],
                                     x_t[:, ds(sp * 2, 2), 0:D],
                                     psp[:, :, 0:D])
                if not biases_zero:
                    for j in range(2):
                        nc.vector.tensor_add(x_t[:, sp * 2 + j, 0:D],
                                             x_t[:, sp * 2 + j, 0:D], sb["bp"])
            ynA, ynA2 = layernorm_fm(x_t, "2")
            ynAf = ynA.rearrange("p a b -> p (a b)")
            ynA2f = ynA2.rearrange("p a b -> p (a b)")
            hfm = hpool.tile([128, 6, TILE], bf16, tag="hfm", name="hfm")
            for mp in range(3):
                psf = ppb.tile([128, 2, TILE], f32, tag="big", name="psf1")
                for mi in range(2):
                    m = mp * 2 + mi
                    nc.tensor.matmul(psf[:, mi], sb["w10"][:, ts(m, 128)],
                                     ynAf, start=True, stop=False)
                    nc.tensor.matmul(psf[:, mi], sb["w11"][:, ts(m, 128)],
                                     ynA2f[0:64], start=False, stop=True)
                if biases_zero:
                    nc.scalar.activation(hfm[:, ds(mp * 2, 2)], psf,
                                         GELU_FUNC)
                else:
                    for mi in range(2):
                        m = mp * 2 + mi
                        nc.scalar.activation(hfm[:, m], psf[:, mi],
                                             GELU_FUNC,
                                             bias=sb["b1"][:, m:m + 1])
            for sp in range(2):
                psf2 = ppm.tile([128, 2, 256], f32, tag="med", name="psf2")
                for j in range(2):
                    s = sp * 2 + j
                    for m in range(6):
                        nc.tensor.matmul(psf2[:, j, 0:D],
                                         hfm[:, m, ts(s, 128)],
                                         sb["w2m"][:, m],
                                         start=(m == 0), stop=(m == 5))
                nc.vector.tensor_add(x_t[:, ds(sp * 2, 2), 0:D],
                                     x_t[:, ds(sp * 2, 2), 0:D],
                                     psf2[:, :, 0:D])
                if not biases_zero:
                    for j in range(2):
                        nc.vector.tensor_add(x_t[:, sp * 2 + j, 0:D],
                                             x_t[:, sp * 2 + j, 0:D], sb["b2"])

        ngroups = (ntiles + GROUP - 1) // GROUP
        for g in range(ngroups):
            tiles = [g * GROUP + t for t in range(GROUP)
                     if g * GROUP + t < ntiles]
            xts = {}
            for it in tiles:
                x_t = xpool.tile([128, NSUB, DP], f32, tag="x", name="x_t")
                nc.sync.dma_start(
                    out=x_t,
                    in_=x_d[ts(it, TILE)].rearrange("(s p) f -> p s f", p=128))
                xts[it] = x_t
            for li in range(DEPTH):
                sb = W[li]
                qk = {}
                for it in tiles:
                    qk[it] = phase_a(sb, xts[it])
                of = {}
                for it in tiles:
                    qkA, qkB, v_tm = qk[it]
                    of[it] = phase_b(sb, qkA, qkB, v_tm)
                for it in tiles:
                    ofmA, ofmB = of[it]
                    phase_c(sb, xts[it], ofmA, ofmB)
            for it in tiles:
                nc.sync.dma_start(
                    out=y_d[ts(it, TILE)].rearrange("(s p) f -> p s f", p=128),
                    in_=xts[it][:, :, 0:D])

    nc.compile()
    _COMPILED[key] = nc
    return nc


def _ensure_ntff_hook():
    import sys, types
    if "antenv.axon_hooks" in sys.modules:
        return True
    try:
        mod = types.ModuleType("antenv.axon_hooks")
        state = {}
        mod.set_axon_ntff_profile_hook = lambda h: state.__setitem__("h", h)
        mod.get_axon_ntff_profile_hook = lambda: state.get("h")
        sys.modules["antenv.axon_hooks"] = mod
        import antenv
        antenv.axon_hooks = mod
        from trn_agent_boot.trn_boot import _ntff_profile_via_ctypes
        mod.set_axon_ntff_profile_hook(
            _ntff_profile_via_ctypes("/opt/axon/libaxon_pjrt.so"))
        return True
    except Exception as e:  # pragma: no cover
        print(f"NTFF hook shim failed: {e}")
        return False


def _run(inputs, trace=False):
    """Shard, execute on 8 cores, gather. Returns (y_full, exec_time_ns)."""
    from concourse.bass_utils import run_bass_kernel_spmd

    if trace:
        trace = _ensure_ntff_hook()

    layers = _fold_weights(inputs)
    bz = all(
        not np.any(np.asarray(d[k], np.float32))
        for d in layers for k in d if k.startswith(("bqk", "bv", "bp", "b1", "b2")))
    nc = _build_nc(biases_zero=bz)

    x = np.asarray(inputs["x"], np.float32)
    pos = np.asarray(inputs["pos"], np.float32)
    w = int(np.asarray(inputs["w"]))
    order = _scanline_order(pos, w)
    x_ord = np.take_along_axis(x, order[..., None], axis=1)
    shards = np.zeros((NCORES, T, DP), np.float32)
    shards[:, :, 0:D] = x_ord.reshape(NCORES, T, D)

    wmap = {}
    for d in layers:
        wmap.update({k: np.ascontiguousarray(v) for k, v in d.items()})

    in_maps = [{"x": shards[c], **wmap} for c in range(NCORES)]
    res = run_bass_kernel_spmd(nc, in_maps, core_ids=list(range(NCORES)),
                               trace=trace)
    y_ord = np.stack([res.results[c]["y"] for c in range(NCORES)])
    y_ord = y_ord.reshape(B, N, D)
    y = np.empty_like(y_ord)
    np.put_along_axis(y, order[..., None], y_ord, axis=1)
    return y.astype(np.float32), res.exec_time_ns


def kernel(**inputs):
    y, _ = _run(inputs, trace=False)
    return y


# revision 5
# speedup vs baseline: 1.6486x; 1.2273x over previous
"""Trainium2 Bass kernel for nn_BasicLayer (sparse cluster attention, 2 layers).

v2 rewrite of the staged baseline. Same host-side strategy (scanline gather,
8 cores x 8192 tokens, folded weights, token-major fp32 residual, bf16 matmul
operands) with an on-device restructure aimed at engine balance and PE
density:

- All layout flips (LN token-major -> feature-major, P -> P^T) go through the
  DMA xbar transpose engine instead of PE transpose + PSUM copy.
- LN: one batched bn_stats pair, rsqrt via fast-inverse-sqrt bit trick +
  1 Newton step (DVE only, no sqrt table), normalize via dual-scalar
  tensor_scalar producing bf16 directly.
- Scores matmuls read per-head q/k slices in place via tile_position row
  packing (no per-head copies).
- Softmax: exp -> scratch E; P = E * (1/rowsum) only on the diagonal
  64x64 blocks into persistent zeroed P buffers (gpsimd); P^T via DMA
  transpose feeds the O matmuls.
- Supertiles processed in groups of 4 with phase-major ordering per layer so
  the scalar engine's activation-table switches (Exp <-> Gelu) amortize
  across the group.
"""

import os
import numpy as np
import ml_dtypes

# ---- problem constants (hardcoded per contract) ----
B, N, D = 4, 16384, 192
DP = 256
HEADS, DH, CLM = 6, 32, 64
GRID_W = 128
DEPTH = 2
NCORES = 8
T = (B * N) // NCORES                # 8192 tokens per core
SUB = 128
NSUB = 4
TILE = SUB * NSUB                    # 512-token supertile
NTILES = T // TILE                   # 16
GROUP = 4                            # supertiles per phase group
DFF = 768

_COMPILED = {}


def _scanline_order(pos, w):
    ix = np.floor(pos[..., 0]).astype(np.int64)
    iy = np.floor(pos[..., 1]).astype(np.int64)
    key = iy * w + np.where(iy % 2 == 1, w - 1 - ix, ix)
    return np.argsort(key, axis=1, kind="stable")


def _fold_weights(inputs):
    """Fold LN affine + biases into matmul weights (same layout as v1)."""
    bf16 = ml_dtypes.bfloat16
    scale = DH ** -0.5
    layers = []
    for i in range(DEPTH):
        g1 = np.asarray(inputs["ln1_g"][i], np.float64)
        b1 = np.asarray(inputs["ln1_b"][i], np.float64)
        Wqkv = np.asarray(inputs["w_qkv"][i], np.float64)
        bqkv = np.asarray(inputs["b_qkv"][i], np.float64)
        w_eff = g1[:, None] * Wqkv
        b_eff = b1 @ Wqkv + bqkv
        wq = w_eff[:, 0:D] * scale
        bq = b_eff[0:D] * scale
        wk = w_eff[:, D:2 * D]
        bk = b_eff[D:2 * D]
        wv = w_eff[:, 2 * D:3 * D]
        bv = b_eff[2 * D:3 * D]
        wqk = np.concatenate(
            [wq[:, :128], wk[:, :128], wq[:, 128:], wk[:, 128:]], axis=1)
        pad64 = np.zeros(64)
        bqk = np.stack(
            [bq[:128], bk[:128],
             np.concatenate([bq[128:], pad64]),
             np.concatenate([bk[128:], pad64])], axis=1)
        wp = np.asarray(inputs["w_proj"][i], np.float64)
        bp = np.asarray(inputs["b_proj"][i], np.float64)
        g2 = np.asarray(inputs["ln2_g"][i], np.float64)
        b2 = np.asarray(inputs["ln2_b"][i], np.float64)
        W1 = np.asarray(inputs["w_fc1"][i], np.float64)
        w1_eff = g2[:, None] * W1
        b1_eff = b2 @ W1 + np.asarray(inputs["b_fc1"][i], np.float64)
        W2 = np.asarray(inputs["w_fc2"][i], np.float64)
        bfc2 = np.asarray(inputs["b_fc2"][i], np.float64)
        bv_t = np.stack(
            [bv[:128], np.concatenate([bv[128:], np.zeros(64)])], axis=1)
        layers.append({
            f"wqk{i}": wqk.astype(bf16),
            f"bqk{i}": bqk.astype(np.float32),
            f"wv{i}": wv.astype(bf16),
            f"bv{i}": bv_t.astype(np.float32),
            f"wp{i}": wp.astype(bf16),
            f"bp{i}": np.tile(bp.astype(np.float32), (128, 1)),
            f"w1{i}": w1_eff.astype(bf16),
            f"b1{i}": b1_eff.reshape(6, 128).T.copy().astype(np.float32),
            f"w2{i}": W2.astype(bf16),
            f"b2{i}": np.tile(bfc2.astype(np.float32), (128, 1)),
        })
    return layers


def _build_nc(biases_zero=True, ntiles=NTILES):
    key = ("nc", biases_zero, ntiles)
    if key in _COMPILED:
        return _COMPILED[key]

    from contextlib import ExitStack
    import concourse.bass as bass
    import concourse.tile as tile
    from concourse import bacc, mybir
    from concourse.bass import ts, ds

    f32 = mybir.dt.float32
    bf16 = mybir.dt.bfloat16
    i32 = mybir.dt.int32
    AF = mybir.ActivationFunctionType
    OP = mybir.AluOpType

    tok_total = ntiles * TILE

    nc = bacc.Bacc("TRN2", target_bir_lowering=False, debug=False,
                   enable_asserts=False, num_devices=NCORES)

    x_d = nc.dram_tensor("x", [tok_total, DP], f32, kind="ExternalInput").ap()
    y_d = nc.dram_tensor("y", [tok_total, D], f32, kind="ExternalOutput").ap()
    wd = []
    for i in range(DEPTH):
        wd.append({
            "wqk": nc.dram_tensor(f"wqk{i}", [D, 384], bf16, kind="ExternalInput").ap(),
            "bqk": nc.dram_tensor(f"bqk{i}", [128, 4], f32, kind="ExternalInput").ap(),
            "wv": nc.dram_tensor(f"wv{i}", [D, D], bf16, kind="ExternalInput").ap(),
            "bv": nc.dram_tensor(f"bv{i}", [128, 2], f32, kind="ExternalInput").ap(),
            "wp": nc.dram_tensor(f"wp{i}", [D, D], bf16, kind="ExternalInput").ap(),
            "bp": nc.dram_tensor(f"bp{i}", [128, D], f32, kind="ExternalInput").ap(),
            "w1": nc.dram_tensor(f"w1{i}", [D, DFF], bf16, kind="ExternalInput").ap(),
            "b1": nc.dram_tensor(f"b1{i}", [128, 6], f32, kind="ExternalInput").ap(),
            "w2": nc.dram_tensor(f"w2{i}", [DFF, D], bf16, kind="ExternalInput").ap(),
            "b2": nc.dram_tensor(f"b2{i}", [128, D], f32, kind="ExternalInput").ap(),
        })

    with tile.TileContext(nc) as tc, ExitStack() as ctx:
        consts = ctx.enter_context(tc.tile_pool(name="consts", bufs=1))
        xpool = ctx.enter_context(tc.tile_pool(name="xpool", bufs=6))
        lnpool = ctx.enter_context(tc.tile_pool(name="lnpool", bufs=3))
        fmpool = ctx.enter_context(tc.tile_pool(name="fmpool", bufs=4))
        qkpool = ctx.enter_context(tc.tile_pool(name="qkpool", bufs=5))
        apool = ctx.enter_context(tc.tile_pool(name="apool", bufs=3))
        ofpool = ctx.enter_context(tc.tile_pool(name="ofpool", bufs=5))
        hpool = ctx.enter_context(tc.tile_pool(name="hpool", bufs=2))
        stpool = ctx.enter_context(tc.tile_pool(name="stpool", bufs=6))
        ppsc = ctx.enter_context(tc.tile_pool(name="ppsc", bufs=1, space="PSUM"))
        ppm = ctx.enter_context(tc.tile_pool(name="ppm", bufs=4, space="PSUM"))

        # persistent softmax buffers: off-diagonal blocks stay 0 forever
        NPBUF = 2
        p_bufs = []
        for pb_i in range(NPBUF):
            pb = consts.tile([128, 2, HEADS, 128], bf16, name=f"pbuf{pb_i}")
            nc.vector.memset(pb, 0.0)
            p_bufs.append(pb)

        # --- load weights into SBUF once ---
        W = []
        for i in range(DEPTH):
            d = wd[i]
            sb = {}
            sb["wqk0"] = consts.tile([128, 384], bf16, name=f"wqk0{i}")
            sb["wqk1"] = consts.tile([64, 384], bf16, name=f"wqk1{i}")
            nc.sync.dma_start(out=sb["wqk0"], in_=d["wqk"][0:128])
            nc.sync.dma_start(out=sb["wqk1"], in_=d["wqk"][128:192])
            sb["wv0"] = consts.tile([128, D], bf16, name=f"wv0{i}")
            sb["wv1"] = consts.tile([64, D], bf16, name=f"wv1{i}")
            nc.sync.dma_start(out=sb["wv0"], in_=d["wv"][0:128])
            nc.sync.dma_start(out=sb["wv1"], in_=d["wv"][128:192])
            sb["wp0"] = consts.tile([128, D], bf16, name=f"wp0{i}")
            sb["wp1"] = consts.tile([64, D], bf16, name=f"wp1{i}")
            nc.sync.dma_start(out=sb["wp0"], in_=d["wp"][0:128])
            nc.sync.dma_start(out=sb["wp1"], in_=d["wp"][128:192])
            sb["w10"] = consts.tile([128, DFF], bf16, name=f"w10{i}")
            sb["w11"] = consts.tile([64, DFF], bf16, name=f"w11{i}")
            nc.sync.dma_start(out=sb["w10"], in_=d["w1"][0:128])
            nc.sync.dma_start(out=sb["w11"], in_=d["w1"][128:192])
            sb["w2m"] = consts.tile([128, 6, D], bf16, name=f"w2m{i}")
            nc.sync.dma_start(
                out=sb["w2m"],
                in_=d["w2"].rearrange("(m p) n -> p m n", p=128))
            for nm in ("bqk", "bv", "b1", "bp", "b2"):
                shp = {"bqk": [128, 4], "bv": [128, 2], "b1": [128, 6],
                       "bp": [128, D], "b2": [128, D]}[nm]
                sb[nm] = consts.tile(shp, f32, name=f"{nm}{i}")
                nc.sync.dma_start(out=sb[nm], in_=d[nm])
            W.append(sb)

        pair_ctr = [0]
        MAGIC = 0x5F3759DF
        # CoreSim lacks Gelu_apprx_tanh; substitute Tanh for sim-only runs.
        GELU_FUNC = (AF.Tanh if os.environ.get("K_SIM_GELU_TANH") == "1"
                     else AF.Gelu_apprx_tanh)

        def layernorm_fm(x_t, tag):
            """LN on token-major x_t -> feature-major bf16 via DMA transpose.
            Returns fmA [128,4,128] (feats 0:128, cols=tokens) and fmA2
            (feats 128:256; partitions 64:128 are pad)."""
            mv = stpool.tile([128, 4, 6], f32, tag="mv", name="mv")
            mv2 = stpool.tile([128, 4, 2], f32, tag="mv2", name="mv2")
            for s in range(NSUB):
                nc.vector.bn_stats(mv[:, s], x_t[:, s, 0:D])
                nc.vector.bn_aggr(mv2[:, s], mv[:, s])
            var = mv2[:, :, 1]                       # [128, 4] stride 2
            t_i = stpool.tile([128, 4], i32, tag="ti", name="t_i")
            y0 = stpool.tile([128, 4], f32, tag="y0", name="y0")
            zz = stpool.tile([128, 4], f32, tag="zz", name="zz")
            r4 = stpool.tile([128, 4], f32, tag="r4", name="r4")
            nc.vector.tensor_scalar(
                out=t_i, in0=var.bitcast(i32), scalar1=1, scalar2=None,
                op0=OP.logical_shift_right)
            nc.vector.tensor_scalar(
                out=y0.bitcast(i32), in0=t_i, scalar1=MAGIC, scalar2=-1,
                op0=OP.subtract, op1=OP.mult)
            nc.vector.scalar_tensor_tensor(
                out=zz, in0=var, scalar=1e-5, in1=y0,
                op0=OP.add, op1=OP.mult)              # (var+eps)*y0
            nc.vector.tensor_tensor(out=zz, in0=zz, in1=y0, op=OP.mult)
            nc.vector.tensor_scalar(
                out=zz, in0=zz, scalar1=-0.5, scalar2=1.5,
                op0=OP.mult, op1=OP.add)              # 1.5 - 0.5 v y0^2
            nc.vector.tensor_tensor(out=r4, in0=zz, in1=y0, op=OP.mult)

            xn = lnpool.tile([128, 2, NSUB, 128], bf16, tag=f"xn{tag}",
                             name=f"xn{tag}")
            for s in range(NSUB):
                nc.vector.tensor_scalar(
                    out=xn[:, :, s], in0=x_t[:, s].rearrange("p (c f) -> p c f", c=2),
                    scalar1=mv2[:, s, 0:1], scalar2=r4[:, s:s + 1],
                    op0=OP.subtract, op1=OP.mult)
            fm2 = fmpool.tile([128, 2, NSUB, 128], bf16, tag=f"fm{tag}",
                              name=f"fm{tag}")
            nc.sync.dma_start_transpose(out=fm2, in_=xn)
            return fm2[:, 0], fm2[:, 1]

        def phase_a(sb, x_t):
            """LN1 + qkv + v. Returns (qkA, qkB, v_tm)."""
            fmA, fmA2 = layernorm_fm(x_t, "1")
            fmAf = fmA.rearrange("p a b -> p (a b)")
            fmA2f = fmA2.rearrange("p a b -> p (a b)")
            # qkv: m-chunks 0,1 are 128-wide (heads 0-3 q|k), 2,3 are 64-wide
            qkA = qkpool.tile([128, 2, TILE], bf16, tag="qkA", name="qkA")
            qkB = qkpool.tile([64, 2, TILE], bf16, tag="qkB", name="qkB")
            psq = []
            for m in range(2):
                ps = ppm.tile([128, TILE], f32, tag="med", name=f"psqA{m}")
                nc.tensor.matmul(ps, sb["wqk0"][:, ts(m, 128)], fmAf,
                                 start=True, stop=False)
                nc.tensor.matmul(ps, sb["wqk1"][:, ts(m, 128)],
                                 fmA2f[0:64], start=False, stop=True)
                psq.append(ps)
            for m in range(2):
                ps = ppm.tile([64, TILE], f32, tag="med", name=f"psqB{m}")
                nc.tensor.matmul(ps, sb["wqk0"][:, ds(256 + m * 64, 64)],
                                 fmAf, start=True, stop=False)
                nc.tensor.matmul(ps, sb["wqk1"][:, ds(256 + m * 64, 64)],
                                 fmA2f[0:64], start=False, stop=True)
                psq.append(ps)
            for m in range(2):
                if biases_zero:
                    nc.vector.tensor_copy(qkA[:, m], psq[m])
                    nc.vector.tensor_copy(qkB[:, m], psq[2 + m])
                else:
                    nc.scalar.activation(qkA[:, m], psq[m], AF.Identity,
                                         bias=sb["bqk"][:, m:m + 1])
                    nc.scalar.activation(qkB[:, m], psq[2 + m], AF.Identity,
                                         bias=sb["bqk"][0:64, 2 + m:3 + m])
            # v (token-major out), pairs of subs per PSUM tile
            v_tm = qkpool.tile([128, NSUB, D], bf16, tag="vtm", name="v_tm")
            for sp in range(2):
                psv = ppm.tile([128, 2, 256], f32, tag="med", name="psv")
                for j in range(2):
                    s = sp * 2 + j
                    nc.tensor.matmul(psv[:, j, 0:D], fmA[:, s], sb["wv0"],
                                     start=True, stop=False)
                    nc.tensor.matmul(psv[:, j, 0:D], fmA2[0:64, s], sb["wv1"],
                                     start=False, stop=True)
                if biases_zero:
                    nc.scalar.activation(v_tm[:, ds(sp * 2, 2)],
                                         psv[:, :, 0:D], AF.Copy)
                else:
                    for j in range(2):
                        nc.scalar.activation(
                            v_tm[:, sp * 2 + j], psv[:, j, 0:D], AF.Identity,
                            bias=sb["bv"][:, 0:1])
            return qkA, qkB, v_tm

        def phase_b(sb, qkA, qkB, v_tm):
            """Cluster attention. Returns (ofmA, ofmB) feature-major o."""
            ofmA = ofpool.tile([128, TILE], bf16, tag="ofA", name="ofmA")
            ofmB = ofpool.tile([64, TILE], bf16, tag="ofB", name="ofmB")
            for sp in range(2):
                oPp = ppm.tile([128, 2, 256], f32, tag="med", name="oPp")
                P2 = p_bufs[pair_ctr[0] % NPBUF]
                pair_ctr[0] += 1
                pkm2 = apool.tile([128, 2, HEADS, 128], bf16, tag="pkm",
                                  name="pkm")
                for j in range(2):
                    s = sp * 2 + j
                    cols = ts(s, 128)
                    # concurrent row-tiled score MMs must land in distinct
                    # PSUM banks: head h -> bank h%4, col block h//4
                    sc = ppsc.tile([128, 4, 512], f32, tag="sc", name="sc")
                    for h in range(HEADS):
                        if h < 4:
                            qs = qkA[ts(h, 32), 0, cols]
                            ks = qkA[ts(h, 32), 1, cols]
                        else:
                            qs = qkB[ts(h - 4, 32), 0, cols]
                            ks = qkB[ts(h - 4, 32), 1, cols]
                        out = sc[:, h % 4, ds((h // 4) * 128, 128)]
                        nc.tensor.matmul(out, qs, ks,
                                         start=True, stop=True,
                                         tile_position=(32 * (h % 4), 0))
                    E = apool.tile([128, HEADS, 128], bf16, tag="E", name="E")
                    sums = stpool.tile([128, HEADS], f32, tag="sm", name="sums")
                    rsum = stpool.tile([128, HEADS], f32, tag="rs", name="rsum")
                    nc.scalar.activation(E[:, 0:4], sc[:, :, 0:128], AF.Exp)
                    nc.scalar.activation(E[:, 4:6], sc[:, 0:2, 128:256], AF.Exp)
                    nc.vector.reduce_sum(sums[0:64], E[0:64, :, 0:64],
                                         axis=mybir.AxisListType.X)
                    nc.vector.reduce_sum(sums[64:128], E[64:128, :, 64:128],
                                         axis=mybir.AxisListType.X)
                    nc.vector.reciprocal(rsum, sums)
                    P = P2[:, j]
                    for half in range(2):
                        hs = ds(half * 64, 64)
                        rs_half = rsum[ds(half * 64, 64)]
                        rsum_b = bass.AP(tensor=rs_half.tensor,
                                         offset=rs_half.offset,
                                         ap=[*rs_half.ap, [0, 64]])
                        nc.gpsimd.tensor_tensor(
                            out=P[hs, :, hs], in0=E[hs, :, hs],
                            in1=rsum_b, op=OP.mult)
                nc.sync.dma_start_transpose(out=pkm2, in_=P2)
                for j in range(2):
                    s = sp * 2 + j
                    for h in range(HEADS):
                        if h < 4:
                            out = oPp[ts(h, 32), j, 0:128]
                            colpos = h * 32
                        else:
                            out = oPp[ts(h - 4, 32), j, 128:256]
                            colpos = (h - 4) * 32
                        nc.tensor.matmul(out, v_tm[:, s, ts(h, 32)],
                                         pkm2[:, j, h], start=True, stop=True,
                                         tile_position=(0, colpos))
                # copy pair of subs out of PSUM
                c0 = sp * 256
                if biases_zero:
                    nc.vector.tensor_copy(
                        ofmA.rearrange("p (a b) -> p a b", a=NSUB)[:, ds(sp * 2, 2)],
                        oPp[:, :, 0:128])
                    nc.scalar.activation(
                        ofmB.rearrange("p (a b) -> p a b", a=NSUB)[:, ds(sp * 2, 2)],
                        oPp[0:64, :, 128:256], AF.Copy)
                else:
                    for j in range(2):
                        nc.scalar.activation(
                            ofmA[:, ds(c0 + j * 128, 128)], oPp[:, j, 0:128],
                            AF.Identity, bias=sb["bv"][:, 0:1])
                        nc.scalar.activation(
                            ofmB[:, ds(c0 + j * 128, 128)],
                            oPp[0:64, j, 128:256],
                            AF.Identity, bias=sb["bv"][0:64, 1:2])
            return ofmA, ofmB

        def phase_c(sb, x_t, ofmA, ofmB):
            """proj + residual, LN2, MLP + residual."""
            for sp in range(2):
                psp = ppm.tile([128, 2, 256], f32, tag="med", name="psp")
                for j in range(2):
                    s = sp * 2 + j
                    nc.tensor.matmul(psp[:, j, 0:D], ofmA[:, ts(s, 128)],
                                     sb["wp0"], start=True, stop=False)
                    nc.tensor.matmul(psp[:, j, 0:D], ofmB[:, ts(s, 128)],
                                     sb["wp1"], start=False, stop=True)
                nc.vector.tensor_add(x_t[:, ds(sp * 2, 2), 0:D],
                                     x_t[:, ds(sp * 2, 2), 0:D],
                                     psp[:, :, 0:D])
                if not biases_zero:
                    for j in range(2):
                        nc.vector.tensor_add(x_t[:, sp * 2 + j, 0:D],
                                             x_t[:, sp * 2 + j, 0:D], sb["bp"])
            ynA, ynA2 = layernorm_fm(x_t, "2")
            ynAf = ynA.rearrange("p a b -> p (a b)")
            ynA2f = ynA2.rearrange("p a b -> p (a b)")
            hfm = hpool.tile([128, 6, TILE], bf16, tag="hfm", name="hfm")
            for m in range(6):
                psf = ppm.tile([128, TILE], f32, tag="med", name="psf1")
                nc.tensor.matmul(psf, sb["w10"][:, ts(m, 128)],
                                 ynAf, start=True, stop=False)
                nc.tensor.matmul(psf, sb["w11"][:, ts(m, 128)],
                                 ynA2f[0:64], start=False, stop=True)
                if biases_zero:
                    nc.scalar.activation(hfm[:, m], psf, GELU_FUNC)
                else:
                    nc.scalar.activation(hfm[:, m], psf, GELU_FUNC,
                                         bias=sb["b1"][:, m:m + 1])
            for sp in range(2):
                psf2 = ppm.tile([128, 2, 256], f32, tag="med", name="psf2")
                for j in range(2):
                    s = sp * 2 + j
                    for m in range(6):
                        nc.tensor.matmul(psf2[:, j, 0:D],
                                         hfm[:, m, ts(s, 128)],
                                         sb["w2m"][:, m],
                                         start=(m == 0), stop=(m == 5))
                nc.vector.tensor_add(x_t[:, ds(sp * 2, 2), 0:D],
                                     x_t[:, ds(sp * 2, 2), 0:D],
                                     psf2[:, :, 0:D])
                if not biases_zero:
                    for j in range(2):
                        nc.vector.tensor_add(x_t[:, sp * 2 + j, 0:D],
                                             x_t[:, sp * 2 + j, 0:D], sb["b2"])

        ngroups = (ntiles + GROUP - 1) // GROUP
        for g in range(ngroups):
            tiles = [g * GROUP + t for t in range(GROUP)
                     if g * GROUP + t < ntiles]
            xts = {}
            for it in tiles:
                x_t = xpool.tile([128, NSUB, DP], f32, tag="x", name="x_t")
                nc.sync.dma_start(
                    out=x_t,
                    in_=x_d[ts(it, TILE)].rearrange("(s p) f -> p s f", p=128))
                xts[it] = x_t
            for li in range(DEPTH):
                sb = W[li]
                qk = {}
                for it in tiles:
                    qk[it] = phase_a(sb, xts[it])
                of = {}
                for it in tiles:
                    qkA, qkB, v_tm = qk[it]
                    of[it] = phase_b(sb, qkA, qkB, v_tm)
                for it in tiles:
                    ofmA, ofmB = of[it]
                    phase_c(sb, xts[it], ofmA, ofmB)
            for it in tiles:
                nc.sync.dma_start(
                    out=y_d[ts(it, TILE)].rearrange("(s p) f -> p s f", p=128),
                    in_=xts[it][:, :, 0:D])

    nc.compile()
    _COMPILED[key] = nc
    return nc


def _ensure_ntff_hook():
    import sys, types
    if "antenv.axon_hooks" in sys.modules:
        return True
    try:
        mod = types.ModuleType("antenv.axon_hooks")
        state = {}
        mod.set_axon_ntff_profile_hook = lambda h: state.__setitem__("h", h)
        mod.get_axon_ntff_profile_hook = lambda: state.get("h")
        sys.modules["antenv.axon_hooks"] = mod
        import antenv
        antenv.axon_hooks = mod
        from trn_agent_boot.trn_boot import _ntff_profile_via_ctypes
        mod.set_axon_ntff_profile_hook(
            _ntff_profile_via_ctypes("/opt/axon/libaxon_pjrt.so"))
        return True
    except Exception as e:  # pragma: no cover
        print(f"NTFF hook shim failed: {e}")
        return False


def _run(inputs, trace=False):
    """Shard, execute on 8 cores, gather. Returns (y_full, exec_time_ns)."""
    from concourse.bass_utils import run_bass_kernel_spmd

    if trace:
        trace = _ensure_ntff_hook()

    layers = _fold_weights(inputs)
    bz = all(
        not np.any(np.asarray(d[k], np.float32))
        for d in layers for k in d if k.startswith(("bqk", "bv", "bp", "b1", "b2")))
    nc = _build_nc(biases_zero=bz)

    x = np.asarray(inputs["x"], np.float32)
    pos = np.asarray(inputs["pos"], np.float32)
    w = int(np.asarray(inputs["w"]))
    order = _scanline_order(pos, w)
    x_ord = np.take_along_axis(x, order[..., None], axis=1)
    shards = np.zeros((NCORES, T, DP), np.float32)
    shards[:, :, 0:D] = x_ord.reshape(NCORES, T, D)

    wmap = {}
    for d in layers:
        wmap.update({k: np.ascontiguousarray(v) for k, v in d.items()})

    in_maps = [{"x": shards[c], **wmap} for c in range(NCORES)]
    res = run_bass_kernel_spmd(nc, in_maps, core_ids=list(range(NCORES)),
                               trace=trace)
    y_ord = np.stack([res.results[c]["y"] for c in range(NCORES)])
    y_ord = y_ord.reshape(B, N, D)
    y = np.empty_like(y_ord)
    np.put_along_axis(y, order[..., None], y_ord, axis=1)
    return y.astype(np.float32), res.exec_time_ns


def kernel(**inputs):
    y, _ = _run(inputs, trace=False)
    return y
